# revision 2
# baseline (speedup 1.0000x reference)
"""Causal self-attention (B=2, T=2048, E=1024, H=16, D=64) on 8 TRN2 cores.

Sharding: core = (batch, head-group): b = core // 4, heads 4g..4g+3 with
g = core % 4 (data parallel over batch x tensor parallel over heads).
Each core computes qkv projection for its 4 heads, causal attention, and
a partial output projection (its head rows of w_proj). Host sums the 4
partials per batch and adds b_proj.

Device layout (per core, all fp32):
  inputs:  xt [1024, 2048] = x[b].T
           wqk [1024, 512] = [w_q cols | w_k cols] for the 4 heads
           wv  [1024, 256]
           wp  [256, 1024] = w_proj rows for the 4 heads
           masks [128, 4, 512] causal masks for the 4 diagonal positions
  output:  out [2048, 1024] partial projection

All matmuls feed each other without transposes:
  QK^T [512, 2048] = (x @ wqk)^T   via lhsT=wqk chunk, rhs=xt chunk
  V    [2048, 256+ones]            via lhsT=xt chunk, rhs=wv chunk
  S^T  [k,q] tiles                 via lhsT=K^T slice, rhs=Q^T slice
  O^T+sums = [V|1]^T P             via lhsT=v tile, rhs=exp(S^T) tile
  out  [2048,1024]                 via lhsT=attn^T chunk, rhs=wp chunk
attn^T is written over the dead Q^T rows of the QK buffer.
"""

import sys

sys.path.insert(0, "/opt/trn_rl_repo")

import numpy as np

import concourse.bacc as bacc
import concourse.tile as tile
from concourse import mybir
from concourse.bass_utils import run_bass_kernel_spmd

F32 = mybir.dt.float32
Exp = mybir.ActivationFunctionType.Exp

B, T, E = 2, 2048, 1024
H, D = 16, 64
NCORES = 8
HPC = 4          # heads per core
QC = HPC * D     # 256 q cols per core
P = 128

_PROG = None


def _build():
    nc = bacc.Bacc("TRN2", target_bir_lowering=False, debug=False)

    xt_d = nc.dram_tensor("xt", [E, T], F32, kind="ExternalInput")
    wqk_d = nc.dram_tensor("wqk", [E, 2 * QC], F32, kind="ExternalInput")
    wv_d = nc.dram_tensor("wv", [E, QC], F32, kind="ExternalInput")
    wp_d = nc.dram_tensor("wp", [QC, E], F32, kind="ExternalInput")
    mk_d = nc.dram_tensor("masks", [P, 4, 512], F32, kind="ExternalInput")
    out_d = nc.dram_tensor("out", [T, E], F32, kind="ExternalOutput")

    KC = E // P       # 8 contraction chunks over E
    NT = T // P       # 16 T tiles of 128

    with tile.TileContext(nc) as tc:
        with (
            tc.tile_pool(name="persist", bufs=1) as persist,
            tc.tile_pool(name="inp", bufs=1) as inp,
            tc.tile_pool(name="pt", bufs=3) as ptp,
            tc.tile_pool(name="small", bufs=2) as small,
            tc.tile_pool(name="stage", bufs=2) as stg,
            tc.tile_pool(name="big", bufs=2, space="PSUM") as big,
            tc.tile_pool(name="po", bufs=3, space="PSUM") as pop,
        ):
            # ---- persistent sbuf ----
            qk_sb = [persist.tile([P, T], F32, name=f"qk{m}") for m in range(4)]
            v_sb = [persist.tile([P, HPC, D + 1], F32, name=f"v{t}") for t in range(NT)]
            mask_sb = persist.tile([P, 4, 512], F32, name="masks")
            wp_sb = [persist.tile([P, E], F32, name=f"wp{c}") for c in range(2)]

            # ---- input DMAs ----
            xt_sb = [inp.tile([P, T], F32, name=f"xt{c}") for c in range(KC)]
            wqk_sb = [inp.tile([P, 2 * QC], F32, name=f"wqk{c}") for c in range(KC)]
            wv_sb = [inp.tile([P, QC], F32, name=f"wv{c}") for c in range(KC)]
            nc.sync.dma_start(out=mask_sb, in_=mk_d[:])
            for c in range(2):
                nc.sync.dma_start(out=wp_sb[c], in_=wp_d[c * P : (c + 1) * P, :])
            for c in range(KC):
                nc.sync.dma_start(out=wqk_sb[c], in_=wqk_d[c * P : (c + 1) * P, :])
                nc.sync.dma_start(out=wv_sb[c], in_=wv_d[c * P : (c + 1) * P, :])
                nc.sync.dma_start(out=xt_sb[c], in_=xt_d[c * P : (c + 1) * P, :])
            for t in range(NT):
                nc.vector.memset(v_sb[t][:, :, D : D + 1], 1.0)

            def v_group(t):
                ps = big.tile([P, 2, 512], F32, name="ps")
                for c in range(KC):
                    nc.tensor.matmul(
                        ps[:, 0, :QC],
                        lhsT=xt_sb[c][:, t * P : (t + 1) * P],
                        rhs=wv_sb[c],
                        start=(c == 0),
                        stop=(c == KC - 1),
                    )
                nc.vector.tensor_copy(
                    v_sb[t][:, :, 0:D],
                    ps[:, 0, :QC].rearrange("p (h d) -> p h d", h=HPC),
                )

            def qk_group(m, u):
                ps = big.tile([P, 2, 512], F32, name="ps")
                for nl in range(2):
                    for c in range(KC):
                        nc.tensor.matmul(
                            ps[:, nl, :],
                            lhsT=wqk_sb[c][:, m * P : (m + 1) * P],
                            rhs=xt_sb[c][:, (2 * u + nl) * 512 : (2 * u + nl + 1) * 512],
                            start=(c == 0),
                            stop=(c == KC - 1),
                        )
                nc.scalar.copy(
                    qk_sb[m][:, u * 1024 : (u + 1) * 1024],
                    ps.rearrange("p a b -> p (a b)"),
                )

            def head_pair(h, p):
                """Attention for head h, query pair p (q in [1024p, 1024p+1024))."""
                po = h // 2
                off = 64 * (h % 2)
                Q = qk_sb[po][off : off + 64, :]
                K = qk_sb[2 + po][off : off + 64, :]
                qbase = p * 1024
                pso = [pop.tile([D + 1, 512], F32, name="po") for _ in range(2)]
                nk = 8 * p + 8
                for i in range(nk):
                    m0 = i - 8 * p          # diag pos in jj=0, valid 0..3
                    m1 = i - 8 * p - 4      # diag pos in jj=1, valid 0..3
                    ps = big.tile([P, 2, 512], F32, name="ps")
                    psf = ps.rearrange("p a b -> p (a b)")
                    pt = ptp.tile([P, 1024], F32, name="pt")
                    # S^T matmuls (trimmed to causal region)
                    if m0 <= 3:  # jj = 0 needed
                        t0 = max(0, 128 * m0)
                        nc.tensor.matmul(
                            psf[:, t0:512],
                            lhsT=K[:, i * P : (i + 1) * P],
                            rhs=Q[:, qbase + t0 : qbase + 512],
                            start=True,
                            stop=True,
                        )
                    t1 = 512 + max(0, 128 * m1)
                    nc.tensor.matmul(
                        psf[:, t1:1024],
                        lhsT=K[:, i * P : (i + 1) * P],
                        rhs=Q[:, qbase + t1 : qbase + 1024],
                        start=True,
                        stop=True,
                    )
                    # exp over the full computed span (one ACT op)
                    s = max(0, 128 * m0) if m0 <= 3 else t1
                    nc.scalar.activation(pt[:, s:1024], psf[:, s:1024], Exp, scale=0.125)
                    # causal masks on diagonal tiles
                    if 0 <= m0 <= 3:
                        nc.vector.tensor_mul(
                            pt[:, 128 * m0 : 512],
                            pt[:, 128 * m0 : 512],
                            mask_sb[:, m0, 128 * m0 : 512],
                        )
                    if 0 <= m1:
                        nc.vector.tensor_mul(
                            pt[:, 512 + 128 * m1 : 1024],
                            pt[:, 512 + 128 * m1 : 1024],
                            mask_sb[:, m1, 128 * m1 : 512],
                        )
                    # PV (+ones row -> denominators)
                    if m0 <= 3:
                        t0 = max(0, 128 * m0)
                        nc.tensor.matmul(
                            pso[0][:, t0:512],
                            lhsT=v_sb[i][:, h, :],
                            rhs=pt[:, t0:512],
                            start=(i == 0),
                            stop=(i == 8 * p + 3),
                        )
                    t1 = max(0, 128 * m1)
                    nc.tensor.matmul(
                        pso[1][:, t1:512],
                        lhsT=v_sb[i][:, h, :],
                        rhs=pt[:, 512 + t1 : 1024],
                        start=(i == 0),
                        stop=(i == nk - 1),
                    )
                # normalize and write attn^T over the dead Q rows
                for jj in range(2):
                    rec = small.tile([1, 512], F32, name="rec")
                    rb = small.tile([64, 512], F32, name="rb")
                    nc.vector.reciprocal(rec, pso[jj][D : D + 1, :])
                    nc.gpsimd.partition_broadcast(rb, rec)
                    nc.vector.tensor_mul(
                        qk_sb[po][off : off + 64, qbase + jj * 512 : qbase + (jj + 1) * 512],
                        pso[jj][0:D, :],
                        rb,
                    )

            # ---- emission order: overlap qkv with attention ----
            for t in range(8):
                v_group(t)
            qk_group(0, 0)
            qk_group(2, 0)
            head_pair(0, 0)
            for t in range(8, NT):
                v_group(t)
            qk_group(0, 1)
            qk_group(2, 1)
            head_pair(0, 1)
            head_pair(1, 0)
            head_pair(1, 1)
            qk_group(1, 0)
            qk_group(3, 0)
            head_pair(2, 0)
            qk_group(1, 1)
            qk_group(3, 1)
            head_pair(2, 1)
            head_pair(3, 0)
            head_pair(3, 1)

            # ---- output projection ----
            for t in range(NT):
                ps = big.tile([P, 2, 512], F32, name="ps")
                for nl in range(2):
                    for c in range(2):
                        nc.tensor.matmul(
                            ps[:, nl, :],
                            lhsT=qk_sb[c][:, t * P : (t + 1) * P],
                            rhs=wp_sb[c][:, nl * 512 : (nl + 1) * 512],
                            start=(c == 0),
                            stop=(c == 1),
                        )
                st = stg.tile([P, 1024], F32, name="st")
                nc.vector.tensor_copy(st, ps.rearrange("p a b -> p (a b)"))
                nc.sync.dma_start(out=out_d[t * P : (t + 1) * P, :], in_=st)

    nc.compile()
    return nc


def _get_prog():
    global _PROG
    if _PROG is None:
        _PROG = _build()
    return _PROG


def _masks_np():
    kk = np.arange(P)[:, None]
    qq = np.arange(512)[None, :]
    return np.stack(
        [((128 * m + kk) <= qq) for m in range(4)], axis=1
    ).astype(np.float32)


def _shard(x, w_qkv, w_proj):
    masks = _masks_np()
    in_maps = []
    for core in range(NCORES):
        b, g = core // HPC, core % HPC
        c0 = g * QC
        in_maps.append(
            {
                "xt": np.ascontiguousarray(x[b].T),
                "wqk": np.ascontiguousarray(
                    np.concatenate(
                        [w_qkv[:, c0 : c0 + QC], w_qkv[:, E + c0 : E + c0 + QC]],
                        axis=1,
                    )
                ),
                "wv": np.ascontiguousarray(w_qkv[:, 2 * E + c0 : 2 * E + c0 + QC]),
                "wp": np.ascontiguousarray(w_proj[c0 : c0 + QC, :]),
                "masks": masks,
            }
        )
    return in_maps


def _run(inputs, **kwargs):
    x = np.asarray(inputs["x"], dtype=np.float32)
    w_qkv = np.asarray(inputs["w_qkv"], dtype=np.float32)
    w_proj = np.asarray(inputs["w_proj"], dtype=np.float32)
    b_proj = np.asarray(inputs["b_proj"], dtype=np.float32)

    nc = _get_prog()
    in_maps = _shard(x, w_qkv, w_proj)
    res = run_bass_kernel_spmd(nc, in_maps, core_ids=list(range(NCORES)), **kwargs)

    out = np.zeros((B, T, E), dtype=np.float32)
    for core in range(NCORES):
        out[core // HPC] += res.results[core]["out"]
    out += b_proj[None, None, :]
    return out, res


def kernel(**inputs):
    out, _ = _run(inputs)
    return out
